# revision 8
# baseline (speedup 1.0000x reference)
"""LSEP loss kernel for Trainium2 (8 NeuronCores, SPMD data-parallel).

loss = log1p( sum_i [ (sum_{c: t=0} exp(x_ic)) * (sum_{c: t=1} exp(-x_ic)) ] )

Strategy: shard the batch (32768) across 8 cores (4096 rows each).
Per core, per [128, 4000] tile (4 samples per partition):
  a = x - BIG*t           (one DVE scalar_tensor_tensor op, int32 t cast on read)
  s_neg_row = sum exp(a)          -> exact exp(x) where t==0, ~0 where t==1
  s_pos_row = sum exp(-a - BIG)   -> exp(-x) where t==1, ~0 where t==0
(both exps via ScalarE activation with free affine + accum_out row reduction)
Then prod = s_neg*s_pos per sample, reduce, DMA [128,1] partial per core,
final scalar sum + log1p on host.
"""

import numpy as np

BATCH = 32768
C = 1000
N_CORES = 8
ROWS = BATCH // N_CORES          # 4096 rows per core
P = 128                          # SBUF partitions
SPP = 4                          # samples per partition per tile
W = SPP * C                      # 4000 free-dim elements per tile
TILE_ROWS = P * SPP              # 512
N_TILES = ROWS // TILE_ROWS      # 8
NSLC = N_TILES * SPP             # 32 accumulated sample-columns
BIG = 50.0

_CACHE = {}


def _build_nc():
    import concourse.bacc as bacc
    import concourse.mybir as mybir
    from concourse.tile import TileContext

    f32 = mybir.dt.float32
    i32 = mybir.dt.int32
    Exp = mybir.ActivationFunctionType.Exp
    Alu = mybir.AluOpType

    nc = bacc.Bacc()
    x = nc.declare_dram_parameter("input", [ROWS, C], f32, isOutput=False)
    t = nc.declare_dram_parameter("target", [ROWS, C], i32, isOutput=False)
    out = nc.declare_dram_parameter("partial", [P, 1], f32, isOutput=True)

    # row r = n*512 + p*4 + s ; free dim packs (s, c) contiguously
    xv = x.rearrange("(n p s) c -> n p (s c)", p=P, s=SPP)
    tv = t.rearrange("(n p s) c -> n p (s c)", p=P, s=SPP)

    with TileContext(nc) as tc:
        with (
            tc.tile_pool(name="io", bufs=3) as io,
            tc.tile_pool(name="acc", bufs=1) as accp,
            tc.tile_pool(name="ps", bufs=1, space="PSUM") as psp,
        ):
            sn = accp.tile([P, NSLC], f32)
            sp = accp.tile([P, NSLC], f32)
            escr = psp.tile([P, C], f32)  # ACT main output scratch (discarded)
            bneg = accp.tile([P, 1], f32)  # bias AP holding -BIG
            nc.vector.memset(bneg[:], -BIG)
            for i in range(N_TILES):
                xt = io.tile([P, W], f32, tag="x")
                tt = io.tile([P, W], i32, tag="t")
                at = io.tile([P, W], f32, tag="a")
                nc.sync.dma_start(xt[:], xv[i])
                nc.sync.dma_start(tt[:], tv[i])
                # a = (t * -BIG) + x
                nc.vector.scalar_tensor_tensor(
                    at[:], tt[:], -BIG, xt[:], op0=Alu.mult, op1=Alu.add
                )
                for s in range(SPP):
                    k = i * SPP + s
                    seg = at[:, s * C : (s + 1) * C]
                    # s_neg: exp(a); masked (t==1) entries exp(x-50) ~ 0
                    nc.scalar.activation(
                        escr[:], seg, Exp, accum_out=sn[:, k : k + 1]
                    )
                    # s_pos: exp(-a-50); masked (t==0) entries exp(-x-50) ~ 0
                    nc.scalar.activation(
                        escr[:], seg, Exp, scale=-1.0, bias=bneg[:],
                        accum_out=sp[:, k : k + 1],
                    )
            prod = accp.tile([P, NSLC], f32)
            tot = accp.tile([P, 1], f32)
            nc.vector.tensor_tensor(prod[:], sn[:], sp[:], Alu.mult)
            nc.vector.reduce_sum(tot[:], prod[:], axis=mybir.AxisListType.X)
            nc.sync.dma_start(out[:], tot[:])
    # Bacc.compile() legalizes sync waits (ISA allows 1 wait/instruction;
    # extra waits become standalone EventSemaphore instructions).
    nc.compile()
    return nc


def _get_nc():
    if "nc" not in _CACHE:
        _CACHE["nc"] = _build_nc()
    return _CACHE["nc"]


def kernel(input, target):
    from concourse.bass_utils import run_bass_kernel_spmd

    x = np.ascontiguousarray(np.asarray(input, dtype=np.float32))
    t = np.ascontiguousarray(np.asarray(target, dtype=np.int32))
    assert x.shape == (BATCH, C) and t.shape == (BATCH, C)

    nc = _get_nc()
    in_maps = [
        {
            "input": x[i * ROWS : (i + 1) * ROWS],
            "target": t[i * ROWS : (i + 1) * ROWS],
        }
        for i in range(N_CORES)
    ]
    res = run_bass_kernel_spmd(nc, in_maps, list(range(N_CORES)))
    total = 0.0
    for r in res.results:
        total += float(np.sum(r["partial"].astype(np.float64)))
    return np.asarray([np.log1p(total)], dtype=np.float32)


# revision 9
# speedup vs baseline: 1.0966x; 1.0966x over previous
"""LSEP loss kernel for Trainium2 (8 NeuronCores, SPMD data-parallel).

loss = log1p( sum_i [ (sum_{c: t=0} exp(x_ic)) * (sum_{c: t=1} exp(-x_ic)) ] )

Strategy: shard the batch (32768) across 8 cores (4096 rows each).
Per core, view the shard as [128 partitions, 32 samples x 1000 classes] and
stream column-chunks:
  a = x - BIG*t           (one DVE scalar_tensor_tensor op, int32 t cast on read)
  s_neg_row = sum exp(a)          -> exact exp(x) where t==0, ~0 where t==1
  s_pos_row = sum exp(-a - BIG)   -> exp(-x) where t==1, ~0 where t==0
(both exps via ScalarE activation free affine + accum_out row reduction into
PSUM accumulators). Epilogue: prod = s_neg*s_pos per sample, reduce,
DMA [128,1] partial per core; final scalar sum + log1p on host.

Chunk schedule [1,1,2,2,...]: small first chunks cut the pipeline ramp-in
(first EXP can start after 0.5 MB x2 instead of 2 MB x2).
"""

import numpy as np

BATCH = 32768
C = 1000
N_CORES = 8
ROWS = BATCH // N_CORES          # 4096 rows per core
P = 128                          # SBUF partitions
SPR = ROWS // P                  # 32 samples per partition
NSLC = SPR                       # accumulated sample-columns per partition
BIG = 50.0
CHUNKS = [1, 1] + [2] * 15       # sample-columns per chunk; sum == 32

_CACHE = {}


def _build_nc():
    import concourse.bacc as bacc
    import concourse.mybir as mybir
    from concourse.tile import TileContext

    f32 = mybir.dt.float32
    i32 = mybir.dt.int32
    Exp = mybir.ActivationFunctionType.Exp
    Alu = mybir.AluOpType

    assert sum(CHUNKS) == NSLC
    wmax = max(CHUNKS) * C

    nc = bacc.Bacc()
    x = nc.declare_dram_parameter("input", [ROWS, C], f32, isOutput=False)
    t = nc.declare_dram_parameter("target", [ROWS, C], i32, isOutput=False)
    out = nc.declare_dram_parameter("partial", [P, 1], f32, isOutput=True)

    # partition p holds samples [p*32, (p+1)*32), 32000 contiguous floats
    xv = x.rearrange("(p s) c -> p (s c)", p=P)
    tv = t.rearrange("(p s) c -> p (s c)", p=P)

    with TileContext(nc) as tc:
        with (
            tc.tile_pool(name="io", bufs=4) as io,
            tc.tile_pool(name="acc", bufs=1) as accp,
            tc.tile_pool(name="ps", bufs=1, space="PSUM") as psp,
        ):
            sn = psp.tile([P, NSLC], f32)
            sp = psp.tile([P, NSLC], f32)
            escr = psp.tile([P, C], f32)  # ACT main output scratch (discarded)
            bneg = accp.tile([P, 1], f32)  # bias AP holding -BIG
            nc.vector.memset(bneg[:], -BIG)
            off = 0
            for ncols in CHUNKS:
                w = ncols * C
                xt = io.tile([P, wmax], f32, tag="x")
                tt = io.tile([P, wmax], i32, tag="t")
                at = io.tile([P, wmax], f32, tag="a")
                nc.sync.dma_start(xt[:, :w], xv[:, off * C : off * C + w])
                nc.sync.dma_start(tt[:, :w], tv[:, off * C : off * C + w])
                # a = (t * -BIG) + x
                nc.vector.scalar_tensor_tensor(
                    at[:, :w], tt[:, :w], -BIG, xt[:, :w],
                    op0=Alu.mult, op1=Alu.add,
                )
                for j in range(ncols):
                    k = off + j
                    seg = at[:, j * C : (j + 1) * C]
                    # s_neg: exp(a); masked (t==1) entries exp(x-50) ~ 0
                    nc.scalar.activation(
                        escr[:], seg, Exp, accum_out=sn[:, k : k + 1]
                    )
                    # s_pos: exp(-a-50); masked (t==0) entries exp(-x-50) ~ 0
                    nc.scalar.activation(
                        escr[:], seg, Exp, scale=-1.0, bias=bneg[:],
                        accum_out=sp[:, k : k + 1],
                    )
                off += ncols
            # epilogue: prod per sample-column, reduce, write [128,1] partial
            sns = accp.tile([P, NSLC], f32)
            prod = accp.tile([P, NSLC], f32)
            tot = accp.tile([P, 1], f32)
            nc.vector.tensor_copy(sns[:], sn[:])
            nc.vector.tensor_tensor(prod[:], sns[:], sp[:], Alu.mult)
            nc.vector.reduce_sum(tot[:], prod[:], axis=mybir.AxisListType.X)
            nc.sync.dma_start(out[:], tot[:])
    # Bacc.compile() legalizes sync waits (ISA allows 1 wait/instruction;
    # extra waits become standalone EventSemaphore instructions).
    nc.compile()
    return nc


def _get_nc():
    if "nc" not in _CACHE:
        _CACHE["nc"] = _build_nc()
    return _CACHE["nc"]


def kernel(input, target):
    from concourse.bass_utils import run_bass_kernel_spmd

    x = np.ascontiguousarray(np.asarray(input, dtype=np.float32))
    t = np.ascontiguousarray(np.asarray(target, dtype=np.int32))
    assert x.shape == (BATCH, C) and t.shape == (BATCH, C)

    nc = _get_nc()
    in_maps = [
        {
            "input": x[i * ROWS : (i + 1) * ROWS],
            "target": t[i * ROWS : (i + 1) * ROWS],
        }
        for i in range(N_CORES)
    ]
    res = run_bass_kernel_spmd(nc, in_maps, list(range(N_CORES)))
    total = 0.0
    for r in res.results:
        total += float(np.sum(r["partial"].astype(np.float64)))
    return np.asarray([np.log1p(total)], dtype=np.float32)
